# revision 4
# baseline (speedup 1.0000x reference)
"""Cross-attention (B=8, C=128, x 64x64 tokens, y 32x32 tokens) on 8 TRN2 cores.

Strategy: data-parallel over batch B (1 batch element per NeuronCore).
Per core, everything is kept in "channels on partitions" layout so no
on-chip transposes are needed:

  xT = x[b] viewed as [C=128, N=4096]      (natural layout of NCHW)
  yT = y[b] viewed as [C=128, M=1024]
  QT[d, n] = (Wq @ xT)[d, n] + bq[d]          matmul, lhsT = Wq^T (host-prep)
  KT[d, m] = (Wk @ yT)[d, m] + bk[d]
  V[m, d]  = (yT^T @ Wv^T)[m, d] + bv[d]      matmul, lhsT = yT slice
  ST[m, n] = sum_d KT[d, m] QT[d, n]          scores^T, m on partitions
  PT[m, n] = exp(scale * ST[m, n])            no max-subtraction: |scale*ST|<~6
  zT[d, n] = sum_m V[m, d] PT[m, n]           accumulated over m tiles in PSUM
  rs[:, n] = sum_m PT[m, n]                   ones-matmul (broadcast over parts)
  out[d,n] = xT[d, n] + zT[d, n] / rs[:, n]

The max-subtraction skip is safe here: scores*scale ~ N(0,1) (x,y ~ N(0,1),
W ~ N(0,1)/sqrt(C)), so exp() stays within ~e^10 of 1.0 -- far inside fp32
range -- and the result is mathematically identical to softmax.
"""

import os

import numpy as np

import concourse.bass as bass  # noqa: F401  (bass types used via tile/bacc)
import concourse.mybir as mybir
import concourse.tile as tile
from concourse import bacc
from concourse.bass_utils import run_bass_kernel_spmd

B = 8
C = 128
N = 64 * 64   # 4096 x-tokens per batch element
M = 32 * 32   # 1024 y-tokens per batch element
NCHUNK = 512  # psum-bank sized n chunk
NCH = N // NCHUNK  # 8
MT = M // 128      # 8 m tiles
SCALE = 1.0 / float(np.sqrt(C))
FP = mybir.dt.float32

# The three big matmul streams (scores^T, z^T, rowsum) use float32r
# operands: 1 col/cycle on the PE (vs 4 for float32) at free-dim >= 256.
# Walrus requires fp32r operands to be *produced* rounded, so the tiles
# feeding those matmuls (QT/KT/V/pt/ones_sq) are declared float32r and get
# rounded for free by their ACT/DVE producer ops. The small projection
# matmuls stay plain fp32 (their inputs come straight from DMA).
FPR = mybir.dt.float32r


def _build():
    nc = bacc.Bacc("TRN2", target_bir_lowering=False, debug=False, num_devices=B)

    x_d = nc.dram_tensor("x", [C, N], FP, kind="ExternalInput").ap()
    y_d = nc.dram_tensor("y", [C, M], FP, kind="ExternalInput").ap()
    wqT_d = nc.dram_tensor("wqT", [C, C], FP, kind="ExternalInput").ap()
    wkT_d = nc.dram_tensor("wkT", [C, C], FP, kind="ExternalInput").ap()
    wvT_d = nc.dram_tensor("wvT", [C, C], FP, kind="ExternalInput").ap()
    bq_d = nc.dram_tensor("bq", [C, 1], FP, kind="ExternalInput").ap()
    bk_d = nc.dram_tensor("bk", [C, 1], FP, kind="ExternalInput").ap()
    bv_d = nc.dram_tensor("bv", [1, C], FP, kind="ExternalInput").ap()
    out_d = nc.dram_tensor("out", [C, N], FP, kind="ExternalOutput").ap()

    with tile.TileContext(nc) as tc:
        with (
            tc.tile_pool(name="const", bufs=1) as cpool,
            tc.tile_pool(name="work", bufs=4) as wpool,
            tc.tile_pool(name="ps_work", bufs=3, space="PSUM") as ps_work,
            tc.tile_pool(name="ps_acc", bufs=2, space="PSUM") as ps_acc,
        ):
            xT = cpool.tile([C, N], FP)
            yT = cpool.tile([C, M], FP)
            wqT = cpool.tile([C, C], FP)
            wkT = cpool.tile([C, C], FP)
            wvT = cpool.tile([C, C], FP)
            bq = cpool.tile([C, 1], FP)
            bk = cpool.tile([C, 1], FP)
            bv_row = cpool.tile([1, C], FP)
            ones_col = cpool.tile([1, C], FP)
            ones_sq = cpool.tile([C, C], FPR)
            zero_bias = cpool.tile([C, 1], FP)
            QT = cpool.tile([C, N], FPR)
            KT = cpool.tile([C, M], FPR)
            V = cpool.tile([C, M], FPR)  # slice mt: [m_local=128, d=128]

            ones_f = cpool.tile([C, C], FP)
            nc.gpsimd.memset(ones_col[:], 1.0)
            nc.gpsimd.memset(ones_f[:], 1.0)
            nc.vector.tensor_copy(ones_sq[:], ones_f[:])
            nc.gpsimd.memset(zero_bias[:], 0.0)

            # input DMAs; x is chunked so Q projection can start early
            nc.sync.dma_start(yT[:], y_d[:])
            nc.sync.dma_start(wqT[:], wqT_d[:])
            nc.sync.dma_start(wkT[:], wkT_d[:])
            nc.sync.dma_start(wvT[:], wvT_d[:])
            nc.sync.dma_start(bq[:], bq_d[:])
            nc.sync.dma_start(bk[:], bk_d[:])
            nc.sync.dma_start(bv_row[:], bv_d[:])
            for j in range(4):
                sl = slice(j * (N // 4), (j + 1) * (N // 4))
                nc.sync.dma_start(xT[:, sl], x_d[:, sl])

            # K projection: KT[d, m] in two 512-chunks
            for j in range(M // NCHUNK):
                sl = slice(j * NCHUNK, (j + 1) * NCHUNK)
                kps = ps_work.tile([C, NCHUNK], FP, name="kps", tag="pswork")
                nc.tensor.matmul(kps[:], wkT[:], yT[:, sl], start=True, stop=True)
                nc.scalar.add(KT[:, sl], kps[:], bk[:])

            # V projection: V[m, d] per m-tile; bias added via K=1 matmul
            for mt in range(MT):
                msl = slice(mt * 128, (mt + 1) * 128)
                vps = ps_work.tile([C, C], FP, name="vps", tag="pswork")
                nc.tensor.matmul(vps[:], yT[:, msl], wvT[:], start=True, stop=False)
                nc.tensor.matmul(
                    vps[:], ones_col[:], bv_row[:], start=False, stop=True
                )
                nc.vector.tensor_copy(V[:, msl], vps[:])

            # Q projection: QT[d, n] in eight 512-chunks
            for j in range(NCH):
                sl = slice(j * NCHUNK, (j + 1) * NCHUNK)
                qps = ps_work.tile([C, NCHUNK], FP, name="qps", tag="pswork")
                nc.tensor.matmul(qps[:], wqT[:], xT[:, sl], start=True, stop=True)
                nc.scalar.add(QT[:, sl], qps[:], bq[:])

            # attention main loop over n chunks
            for j in range(NCH):
                nsl = slice(j * NCHUNK, (j + 1) * NCHUNK)
                zt = ps_acc.tile([C, NCHUNK], FP, name="zt", tag="zt")
                rs = ps_acc.tile([C, NCHUNK], FP, name="rs", tag="rs")
                for mt in range(MT):
                    msl = slice(mt * 128, (mt + 1) * 128)
                    st = ps_work.tile([C, NCHUNK], FP, name="st", tag="pswork")
                    nc.tensor.matmul(
                        st[:], KT[:, msl], QT[:, nsl], start=True, stop=True
                    )
                    pt = wpool.tile([C, NCHUNK], FPR, name="pt", tag="pt")
                    nc.scalar.activation(
                        pt[:],
                        st[:],
                        mybir.ActivationFunctionType.Exp,
                        bias=zero_bias[:],
                        scale=SCALE,
                    )
                    nc.tensor.matmul(
                        zt[:], V[:, msl], pt[:],
                        start=(mt == 0), stop=(mt == MT - 1),
                    )
                    nc.tensor.matmul(
                        rs[:], ones_sq[:], pt[:],
                        start=(mt == 0), stop=(mt == MT - 1),
                    )
                recip = wpool.tile([C, NCHUNK], FP, name="recip", tag="recip")
                nc.vector.reciprocal(recip[:], rs[:])
                o = wpool.tile([C, NCHUNK], FP, name="o", tag="o")
                nc.vector.tensor_mul(o[:], zt[:], recip[:])
                nc.vector.tensor_add(o[:], o[:], xT[:, nsl])
                nc.sync.dma_start(out_d[:, nsl], o[:])

    nc.compile()
    return nc


_CACHE = {}


def _get_nc():
    if "nc" not in _CACHE:
        _CACHE["nc"] = _build()
    return _CACHE["nc"]


def _make_in_maps(inputs):
    x = np.ascontiguousarray(np.asarray(inputs["x"], np.float32)).reshape(B, C, N)
    y = np.ascontiguousarray(np.asarray(inputs["y"], np.float32)).reshape(B, C, M)
    wqT = np.ascontiguousarray(np.asarray(inputs["Wq"], np.float32).T)
    wkT = np.ascontiguousarray(np.asarray(inputs["Wk"], np.float32).T)
    wvT = np.ascontiguousarray(np.asarray(inputs["Wv"], np.float32).T)
    bq = np.ascontiguousarray(np.asarray(inputs["bq"], np.float32).reshape(C, 1))
    bk = np.ascontiguousarray(np.asarray(inputs["bk"], np.float32).reshape(C, 1))
    bv = np.ascontiguousarray(np.asarray(inputs["bv"], np.float32).reshape(1, C))
    return [
        {
            "x": np.ascontiguousarray(x[b]),
            "y": np.ascontiguousarray(y[b]),
            "wqT": wqT,
            "wkT": wkT,
            "wvT": wvT,
            "bq": bq,
            "bk": bk,
            "bv": bv,
        }
        for b in range(B)
    ]


def _run(inputs, trace=False, **kwargs):
    nc = _get_nc()
    res = run_bass_kernel_spmd(
        nc, _make_in_maps(inputs), list(range(B)), trace=trace, **kwargs
    )
    out = np.stack(
        [np.asarray(res.results[b]["out"], np.float32).reshape(C, 64, 64)
         for b in range(B)]
    )
    return out, res


def kernel(**inputs) -> np.ndarray:
    out, _ = _run(inputs, trace=False)
    return out


if __name__ == "__main__":
    # smoke: build only
    os.environ.setdefault("BASS_NEVER_TRACE", "")
    _get_nc()
    print("build ok")


# revision 5
# speedup vs baseline: 1.1282x; 1.1282x over previous
"""Cross-attention (B=8, C=128, x 64x64 tokens, y 32x32 tokens) on 8 TRN2 cores.

Strategy: data-parallel over batch B (1 batch element per NeuronCore).
Per core, everything is kept in "channels on partitions" layout so no
on-chip transposes are needed:

  xT = x[b] viewed as [C=128, N=4096]      (natural layout of NCHW)
  yT = y[b] viewed as [C=128, M=1024]
  QT[d, n] = (Wq @ xT)[d, n] + bq[d]          matmul, lhsT = Wq^T (host-prep)
  KT[d, m] = (Wk @ yT)[d, m] + bk[d]
  V[m, d]  = (yT^T @ Wv^T)[m, d] + bv[d]      matmul, lhsT = yT slice
  ST[m, n] = sum_d KT[d, m] QT[d, n]          scores^T, m on partitions
  PT[m, n] = exp(scale * ST[m, n])            no max-subtraction: |scale*ST|<~6
  zT[d, n] = sum_m V[m, d] PT[m, n]           accumulated over m tiles in PSUM
  rs[:, n] = sum_m PT[m, n]                   ones-matmul (broadcast over parts)
  out[d,n] = xT[d, n] + zT[d, n] / rs[:, n]

The max-subtraction skip is safe here: scores*scale ~ N(0,1) (x,y ~ N(0,1),
W ~ N(0,1)/sqrt(C)), so exp() stays within ~e^10 of 1.0 -- far inside fp32
range -- and the result is mathematically identical to softmax.
"""

import os

import numpy as np

import concourse.bass as bass  # noqa: F401  (bass types used via tile/bacc)
import concourse.mybir as mybir
import concourse.tile as tile
from concourse import bacc
from concourse.bass_utils import run_bass_kernel_spmd

B = 8
C = 128
N = 64 * 64   # 4096 x-tokens per batch element
M = 32 * 32   # 1024 y-tokens per batch element
NCHUNK = 512  # psum-bank sized n chunk
NCH = N // NCHUNK  # 8
MT = M // 128      # 8 m tiles
SCALE = 1.0 / float(np.sqrt(C))
FP = mybir.dt.float32

# The three big matmul streams (scores^T, z^T, rowsum) use float32r
# operands: 1 col/cycle on the PE (vs 4 for float32) at free-dim >= 256.
# Walrus requires fp32r operands to be *produced* rounded, so the tiles
# feeding those matmuls (QT/KT/V/pt/ones_sq) are declared float32r and get
# rounded for free by their ACT/DVE producer ops. The small projection
# matmuls stay plain fp32 (their inputs come straight from DMA).
FPR = mybir.dt.float32r


def _build():
    nc = bacc.Bacc("TRN2", target_bir_lowering=False, debug=False, num_devices=B)

    x_d = nc.dram_tensor("x", [C, N], FPR, kind="ExternalInput").ap()
    y_d = nc.dram_tensor("y", [C, M], FPR, kind="ExternalInput").ap()
    wqT_d = nc.dram_tensor("wqT", [C, C], FPR, kind="ExternalInput").ap()
    wkT_d = nc.dram_tensor("wkT", [C, C], FPR, kind="ExternalInput").ap()
    wvT_d = nc.dram_tensor("wvT", [C, C], FPR, kind="ExternalInput").ap()
    bq_d = nc.dram_tensor("bq", [C, 1], FP, kind="ExternalInput").ap()
    bk_d = nc.dram_tensor("bk", [C, 1], FP, kind="ExternalInput").ap()
    bv_d = nc.dram_tensor("bv", [1, C], FPR, kind="ExternalInput").ap()
    out_d = nc.dram_tensor("out", [C, N], FP, kind="ExternalOutput").ap()

    with tile.TileContext(nc) as tc:
        with (
            tc.tile_pool(name="const", bufs=1) as cpool,
            tc.tile_pool(name="work", bufs=4) as wpool,
            tc.tile_pool(name="ps_work", bufs=3, space="PSUM") as ps_work,
            tc.tile_pool(name="ps_acc", bufs=2, space="PSUM") as ps_acc,
        ):
            xT = cpool.tile([C, N], FPR)
            yT = cpool.tile([C, M], FPR)
            wqT = cpool.tile([C, C], FPR)
            wkT = cpool.tile([C, C], FPR)
            wvT = cpool.tile([C, C], FPR)
            bq = cpool.tile([C, 1], FP)
            bk = cpool.tile([C, 1], FP)
            bv_row = cpool.tile([1, C], FPR)
            ones_col = cpool.tile([1, C], FPR)
            ones_sq = cpool.tile([C, C], FPR)
            zero_bias = cpool.tile([C, 1], FP)
            QT = cpool.tile([C, N], FPR)
            KT = cpool.tile([C, M], FPR)
            V = cpool.tile([C, M], FPR)  # slice mt: [m_local=128, d=128]

            ones_f = cpool.tile([C, C], FP)
            nc.gpsimd.memset(ones_f[:], 1.0)
            nc.vector.tensor_copy(ones_sq[:], ones_f[:])
            nc.vector.tensor_copy(ones_col[:], ones_f[:1, :])
            nc.gpsimd.memset(zero_bias[:], 0.0)

            # input DMAs; y halves and x chunks split across both HWDGE
            # rings (sync + scalar) so descriptor fetch runs in parallel
            nc.sync.dma_start(yT[:, : M // 2], y_d[:, : M // 2])
            nc.scalar.dma_start(yT[:, M // 2 :], y_d[:, M // 2 :])
            nc.sync.dma_start(wkT[:], wkT_d[:])
            nc.scalar.dma_start(wvT[:], wvT_d[:])
            nc.sync.dma_start(wqT[:], wqT_d[:])
            nc.sync.dma_start(bq[:], bq_d[:])
            nc.sync.dma_start(bk[:], bk_d[:])
            nc.scalar.dma_start(bv_row[:], bv_d[:])
            for j in range(4):
                sl = slice(j * (N // 4), (j + 1) * (N // 4))
                eng = nc.sync if j % 2 == 0 else nc.scalar
                eng.dma_start(xT[:, sl], x_d[:, sl])

            # K projection: KT[d, m] in two 512-chunks
            for j in range(M // NCHUNK):
                sl = slice(j * NCHUNK, (j + 1) * NCHUNK)
                kps = ps_work.tile([C, NCHUNK], FP, name="kps", tag="pswork")
                nc.tensor.matmul(kps[:], wkT[:], yT[:, sl], start=True, stop=True)
                nc.scalar.add(KT[:, sl], kps[:], bk[:])

            # V projection: V[m, d] per m-tile; bias added via K=1 matmul
            for mt in range(MT):
                msl = slice(mt * 128, (mt + 1) * 128)
                vps = ps_work.tile([C, C], FP, name="vps", tag="pswork")
                nc.tensor.matmul(vps[:], yT[:, msl], wvT[:], start=True, stop=False)
                nc.tensor.matmul(
                    vps[:], ones_col[:], bv_row[:], start=False, stop=True
                )
                nc.vector.tensor_copy(V[:, msl], vps[:])

            # Q projection: QT[d, n] in eight 512-chunks
            for j in range(NCH):
                sl = slice(j * NCHUNK, (j + 1) * NCHUNK)
                qps = ps_work.tile([C, NCHUNK], FP, name="qps", tag="pswork")
                nc.tensor.matmul(qps[:], wqT[:], xT[:, sl], start=True, stop=True)
                nc.scalar.add(QT[:, sl], qps[:], bq[:])

            # attention main loop over n chunks
            for j in range(NCH):
                nsl = slice(j * NCHUNK, (j + 1) * NCHUNK)
                zt = ps_acc.tile([C, NCHUNK], FP, name="zt", tag="zt")
                rs = ps_acc.tile([C, NCHUNK], FP, name="rs", tag="rs")
                for mt in range(MT):
                    msl = slice(mt * 128, (mt + 1) * 128)
                    st = ps_work.tile([C, NCHUNK], FP, name="st", tag="pswork")
                    nc.tensor.matmul(
                        st[:], KT[:, msl], QT[:, nsl], start=True, stop=True
                    )
                    pt = wpool.tile([C, NCHUNK], FPR, name="pt", tag="pt")
                    nc.scalar.activation(
                        pt[:],
                        st[:],
                        mybir.ActivationFunctionType.Exp,
                        bias=zero_bias[:],
                        scale=SCALE,
                    )
                    nc.tensor.matmul(
                        zt[:], V[:, msl], pt[:],
                        start=(mt == 0), stop=(mt == MT - 1),
                    )
                    nc.tensor.matmul(
                        rs[:], ones_sq[:], pt[:],
                        start=(mt == 0), stop=(mt == MT - 1),
                    )
                recip = wpool.tile([C, NCHUNK], FP, name="recip", tag="recip")
                nc.vector.reciprocal_approx_fast(recip[:], rs[:])
                o = wpool.tile([C, NCHUNK], FP, name="o", tag="o")
                nc.vector.tensor_mul(o[:], zt[:], recip[:])
                nc.vector.tensor_add(o[:], o[:], xT[:, nsl].bitcast(FP))
                oeng = nc.sync if j % 2 == 0 else nc.scalar
                oeng.dma_start(out_d[:, nsl], o[:])

    nc.compile()
    return nc


_CACHE = {}


def _get_nc():
    if "nc" not in _CACHE:
        _CACHE["nc"] = _build()
    return _CACHE["nc"]


def _make_in_maps(inputs):
    x = np.ascontiguousarray(np.asarray(inputs["x"], np.float32)).reshape(B, C, N)
    y = np.ascontiguousarray(np.asarray(inputs["y"], np.float32)).reshape(B, C, M)
    wqT = np.ascontiguousarray(np.asarray(inputs["Wq"], np.float32).T)
    wkT = np.ascontiguousarray(np.asarray(inputs["Wk"], np.float32).T)
    wvT = np.ascontiguousarray(np.asarray(inputs["Wv"], np.float32).T)
    bq = np.ascontiguousarray(np.asarray(inputs["bq"], np.float32).reshape(C, 1))
    bk = np.ascontiguousarray(np.asarray(inputs["bk"], np.float32).reshape(C, 1))
    bv = np.ascontiguousarray(np.asarray(inputs["bv"], np.float32).reshape(1, C))
    return [
        {
            "x": np.ascontiguousarray(x[b]),
            "y": np.ascontiguousarray(y[b]),
            "wqT": wqT,
            "wkT": wkT,
            "wvT": wvT,
            "bq": bq,
            "bk": bk,
            "bv": bv,
        }
        for b in range(B)
    ]


def _run(inputs, trace=False, **kwargs):
    nc = _get_nc()
    res = run_bass_kernel_spmd(
        nc, _make_in_maps(inputs), list(range(B)), trace=trace, **kwargs
    )
    out = np.stack(
        [np.asarray(res.results[b]["out"], np.float32).reshape(C, 64, 64)
         for b in range(B)]
    )
    return out, res


def kernel(**inputs) -> np.ndarray:
    out, _ = _run(inputs, trace=False)
    return out


if __name__ == "__main__":
    # smoke: build only
    os.environ.setdefault("BASS_NEVER_TRACE", "")
    _get_nc()
    print("build ok")
